# revision 27
# baseline (speedup 1.0000x reference)
"""Multi-head attention (B=2, N=2048, C=1024, H=16) on 8 trn2 NeuronCores.

Tensor-parallel over heads: core c computes heads {2c, 2c+1} for both batch
elements and emits a partial output y_c = attn_out_c @ W_out[local rows];
the host sums the 8 partials and adds b_out.

Per-core pipeline (single TileContext, ~3k instructions, fully unrolled):
  - x^T is loaded once into SBUF (bf16, host pre-transposed/pre-tiled so
    every DMA is a contiguous [128, 512] block; ~600ns trigger cost per DMA
    makes few-and-large transfers matter).
  - QKV^T projection with stacked per-head weights ([128, 128] stationary =
    both heads), biases folded into the PSUM->SBUF eviction.
  - S^T = K @ Q^T with both local heads in full-K=128 matmuls via a
    zero-padded Q^T layout (QP block h holds Q^T_h in its own 64 rows,
    zeros in the other 64: K=64 matmuls measured 424ns vs 222ns for K=128,
    so padding the contraction is a 2x win).
  - P^T = exp(S^T / 32) on ScalarE straight from PSUM ([128, 1024] ops;
    no max-subtraction needed: |scores|/32 <~ 1.5).
  - PV via ones-augmented V (65th stationary column accumulates the softmax
    denominators for free). V is transposed on the PE (128x128 tiles).
  - Normalization: evict PV PSUM, ones-broadcast matmul of the sums row,
    reciprocal_approx_fast (18-bit), DVE multiply -> out^T in fp32r.
  - Output projection in fp32r (fp32 rounded to 11 mantissa bits -- full PE
    rate at free-dim >= 256) with direct DMA of y tiles.

Emission order IS program order for Tile: batch-1 QKV projection units and
deferred projection stores are injected inside batch-0's attention loop
(after their producers) so the static per-engine schedule keeps the PE fed
during exp-gated stretches. Never emit a consumer before its producer:
reads of not-yet-written SBUF regions silently bind to stale contents.

Measured: ~245us on-device (max over 8 cores), PE-bound (~200us PE busy);
absmax error ~4e-3 of output scale vs the fp32 reference (bf16 operand
rounding; projection and normalization run at fp32r precision).
"""
import sys

sys.path.insert(0, "/opt/trn_rl_repo")

import ml_dtypes
import numpy as np

import concourse.bacc as bacc
import concourse.mybir as mybir
import concourse.tile as tile
from concourse import bass_utils
from concourse.masks import make_identity

F32 = mybir.dt.float32
F32R = mybir.dt.float32r
BF16 = mybir.dt.bfloat16
NPBF16 = ml_dtypes.bfloat16

EMB = 1024
HEADS = 16
B = 2
SEQ = 2048
D = 64
NCORES = 8
HPC = HEADS // NCORES          # heads per core = 2
LD = HPC * D                   # local head dim = 128
TSEQ = B * SEQ                 # 4096
CC = EMB // 128                # contraction chunks = 8
SCALE = float(EMB) ** -0.5     # 1/32

QCH = 512                      # q chunk (free dim of S^T matmuls)
NQ = SEQ // QCH                # 4 q-chunks per batch
NK = SEQ // 128                # 16 k-chunks per batch


def _round_fp32r(x: np.ndarray) -> np.ndarray:
    bits = np.ascontiguousarray(x, dtype=np.float32).view(np.uint32)
    out = ((bits.astype(np.uint64) + 0x800) & 0xFFFFF000).astype(np.uint32)
    return out.view(np.float32)


def _build():
    nc = bacc.Bacc("TRN2", target_bir_lowering=False, debug=False,
                   num_devices=NCORES)

    xT = nc.dram_tensor("xT", [CC, 128, TSEQ], BF16, kind="ExternalInput")
    wqkv = nc.dram_tensor("wqkv", [128, CC * 3 * LD], BF16,
                          kind="ExternalInput")
    bqkv = nc.dram_tensor("bqkv", [LD, 3], F32, kind="ExternalInput")
    wout = nc.dram_tensor("wout", [LD, EMB], F32R, kind="ExternalInput")
    ones = nc.dram_tensor("ones", [1, D], F32R, kind="ExternalInput")
    onescol = nc.dram_tensor("onescol", [128, 1], F32R, kind="ExternalInput")
    y = nc.dram_tensor("y", [TSEQ // 128, 128, EMB], F32,
                       kind="ExternalOutput")

    xT_c = xT.ap()
    wqkv_c = wqkv.ap()

    with tile.TileContext(nc) as tc:
        with (
            tc.tile_pool(name="persist", bufs=1) as persist,
            tc.tile_pool(name="xt", bufs=2) as xtp,
            tc.tile_pool(name="vt", bufs=2) as vtp,
            tc.tile_pool(name="psb", bufs=6) as psb,
            tc.tile_pool(name="norm", bufs=3) as normp,
            tc.tile_pool(name="yout", bufs=10) as youtp,
            tc.tile_pool(name="ps_st", bufs=2, space="PSUM") as ps_st,
            tc.tile_pool(name="ps_pv", bufs=1, space="PSUM") as ps_pv,
            tc.tile_pool(name="ps_misc", bufs=2, space="PSUM") as ps_misc,
        ):
            # ---- constants / weights ----
            ident = persist.tile([128, 128], BF16, tag="ident")
            make_identity(nc, ident[:])
            ones_sb = persist.tile([1, D], F32R, tag="ones")
            nc.sync.dma_start(ones_sb[:], ones.ap())
            onescol_sb = persist.tile([128, 1], F32R, tag="onescol")
            nc.sync.dma_start(onescol_sb[:], onescol.ap())

            # HAM warm-up: ~3.5us of dummy matmuls on the identity so the PE
            # clock gate opens (K=8/8) while the startup DMAs are in flight.
            wps = ps_misc.tile([128, 128], F32, tag="misc", name="warm")
            for _ in range(32):
                nc.tensor.matmul(wps[:], ident[:], ident[:],
                                 start=True, stop=True)

            bqkv_sb = persist.tile([LD, 3], F32, tag="bqkv")
            nc.sync.dma_start(bqkv_sb[:], bqkv.ap())
            bias_sb = {nm: bqkv_sb[:, i:i + 1]
                       for i, nm in enumerate(("q", "k", "v"))}
            wall = persist.tile([128, CC * 3 * LD], BF16, tag="wall")
            wchunk = 3 * LD
            for kc in range(CC):
                eng = nc.gpsimd if kc % 2 else nc.sync
                eng.dma_start(wall[:, kc * wchunk:(kc + 1) * wchunk],
                              wqkv_c[:, kc * wchunk:(kc + 1) * wchunk])
                # DMA-paced dummy matmuls: each fires when its weight piece
                # lands, keeping the PE activity window busy until real work
                wps2 = ps_misc.tile([128, 128], F32, tag="misc",
                                    name=f"warm{kc}")
                for _ in range(3):
                    nc.tensor.matmul(wps2[:], wall[:, kc * wchunk:
                                                   kc * wchunk + 128],
                                     ident[:], start=True, stop=True)

            xfull = {}

            def load_x_window(w):
                for kc in range(CC):
                    t = persist.tile([128, 512], BF16, tag=f"xf{kc}_{w}",
                                     name=f"xf{kc}_{w}")
                    eng = nc.gpsimd if kc % 2 else nc.sync
                    eng.dma_start(t[:], xT_c[kc, :, w * 512:(w + 1) * 512])
                    xfull[kc, w] = t

            load_x_window(0)
            w_sb = {}
            for kc in range(CC):
                for i, nm in enumerate(("q", "k", "v")):
                    w_sb[nm, kc] = wall[:, (kc * 3 + i) * LD:
                                        (kc * 3 + i + 1) * LD]
            for w in range(1, TSEQ // 512):
                load_x_window(w)
            wout_sb = persist.tile([LD, EMB], F32R, tag="wout")

            # persistent activations (per batch)
            # QP: zero-padded Q^T. Block h occupies cols [h*SEQ, (h+1)*SEQ):
            # rows [h*64,(h+1)*64) hold Q^T_h, the other 64 rows are zero.
            QP = [persist.tile([128, HPC * SEQ], BF16, tag=f"QP{b}",
                               name=f"QP{b}") for b in range(B)]
            KT = [persist.tile([LD, SEQ], BF16, tag=f"KT{b}", name=f"KT{b}")
                  for b in range(B)]
            outT = [persist.tile([LD, SEQ], F32R, tag=f"outT{b}",
                                 name=f"outT{b}") for b in range(B)]
            vaug = {}  # (b, kc) -> [128, 2*(D+1)] tile
            for b in range(B):
                for kc in range(NK):
                    vaug[b, kc] = persist.tile([128, 2 * (D + 1)], BF16,
                                               tag=f"vaug{b}_{kc}",
                                               name=f"vaug{b}_{kc}")

            def phase_a_units(b, scs):
                """QKV^T projection + V transpose for batch b, as a list of
                per-tensor emit closures (~2us of PE work each)."""
                units = []
                for sc in scs:
                    s0 = sc * 512            # batch-local seq offset
                    g0 = b * SEQ + s0        # global column in xT

                    def unit(nm, b=b, s0=s0, g0=g0, sc=sc):
                        ps = ps_misc.tile([128, 512], F32, tag="misc")
                        for kc in range(CC):
                            nc.tensor.matmul(
                                ps[:], w_sb[nm, kc],
                                xfull[kc, g0 // 512][:],
                                start=(kc == 0), stop=(kc == CC - 1))
                        if nm == "q":
                            for h in range(HPC):
                                nc.vector.tensor_scalar_add(
                                    QP[b][h * D:(h + 1) * D,
                                          h * SEQ + s0:h * SEQ + s0 + 512],
                                    ps[h * D:(h + 1) * D, :],
                                    bias_sb["q"][h * D:(h + 1) * D, :])
                        elif nm == "k":
                            nc.vector.tensor_scalar_add(
                                KT[b][:, s0:s0 + 512], ps[:], bias_sb["k"])
                        else:
                            vt = vtp.tile([128, 512], BF16, tag="vt")
                            nc.vector.tensor_scalar_add(vt[:], ps[:],
                                                        bias_sb["v"])
                            for j in range(4):
                                va = vaug[b, sc * 4 + j]
                                pst = ps_misc.tile([128, 128], BF16,
                                                   tag="misc")
                                nc.tensor.transpose(
                                    pst[:], vt[:, j * 128:(j + 1) * 128],
                                    ident[:])
                                nc.vector.tensor_copy(va[:, 0:D],
                                                      pst[:, 0:D])
                                nc.vector.tensor_copy(va[:, D + 1:2 * D + 1],
                                                      pst[:, D:2 * D])
                                nc.vector.tensor_copy(va[:, D:D + 1],
                                                      onescol_sb[:])
                                nc.vector.tensor_copy(
                                    va[:, 2 * D + 1:2 * D + 2], onescol_sb[:])

                    for nm in ("q", "k", "v"):
                        units.append(lambda nm=nm, u=unit: u(nm))
                return units

            def phase_bc(b, fill_units, pre=None):
                """Attention for batch b; fill_units and the previous
                q-chunk's projection are injected inside the kc loop so the
                static per-engine order keeps both PE and ACT fed. `pre`
                maps kc -> producer units that must be emitted before that
                kc group of q-chunk 0 (used to overlap the tail of the
                QKV projection with the start of attention)."""
                fill = list(fill_units)
                fi = 0
                pending = []
                pre = pre or {}

                def proj_unit(b, sc, n, eng=None):
                    rt = b * (SEQ // 128) + sc
                    ps = ps_misc.tile([128, 512], F32, tag="misc")
                    nc.tensor.matmul(
                        ps[:], outT[b][:, sc * 128:(sc + 1) * 128],
                        wout_sb[:, n * 512:(n + 1) * 512],
                        start=True, stop=True)
                    yt = youtp.tile([128, 512], F32, tag="yt")
                    nc.vector.tensor_copy(yt[:], ps[:])
                    if eng is None:
                        eng = nc.gpsimd if (sc + n) % 2 else nc.sync
                    eng.dma_start(
                        y.ap()[rt, :, n * 512:(n + 1) * 512], yt[:])

                for q in range(NQ):
                    q0 = q * QCH
                    pvs = [ps_pv.tile([D + 1, QCH], F32, tag=f"pv{h}",
                                      name=f"pv{h}") for h in range(HPC)]
                    for kc in range(NK):
                        if q == 0:
                            for u in pre.get(kc, ()):
                                u()
                        st = ps_st.tile([128, 2 * QCH], F32, tag="st")
                        k0 = kc * 128
                        for h in range(HPC):
                            nc.tensor.matmul(
                                st[:, h * QCH:(h + 1) * QCH],
                                KT[b][:, k0:k0 + 128],
                                QP[b][:, h * SEQ + q0:h * SEQ + q0 + QCH],
                                start=True, stop=True)
                        pt = psb.tile([128, 2 * QCH], BF16, tag="pt")
                        nc.scalar.activation(pt[:], st[:],
                                             mybir.ActivationFunctionType.Exp,
                                             scale=SCALE)
                        for h in range(HPC):
                            nc.tensor.matmul(
                                pvs[h][:],
                                vaug[b, kc][:, h * (D + 1):(h + 1) * (D + 1)],
                                pt[:, h * QCH:(h + 1) * QCH],
                                start=(kc == 0), stop=(kc == NK - 1))
                        if kc % 2 == 1 and pending:
                            pending.pop(0)()
                        if (q > 0 and kc in (2, 5, 8, 11, 14)
                                and fi < len(fill)):
                            fill[fi]()
                            fi += 1
                    # normalize: out^T[d, q] / colsum -> outT (fp32r).
                    for h in range(HPC):
                        pe = normp.tile([D, QCH], F32R, tag="pe")
                        nc.vector.tensor_copy(pe[:], pvs[h][0:D, :])
                        ss = normp.tile([1, QCH], F32R, tag="ss")
                        nc.vector.tensor_copy(ss[:], pvs[h][D:D + 1, :])
                        bc = ps_misc.tile([D, QCH], F32, tag="misc")
                        nc.tensor.matmul(bc[:], ones_sb[:], ss[:],
                                         start=True, stop=True)
                        rc = normp.tile([D, QCH], F32, tag="rc")
                        nc.vector.reciprocal_approx_fast(rc[:], bc[:])
                        nc.vector.tensor_mul(
                            outT[b][h * D:(h + 1) * D, q0:q0 + QCH],
                            pe[:], rc[:])
                    pending += [
                        (lambda b=b, sc=sc, n=n, eng=None:
                         proj_unit(b, sc, n, eng))
                        for sc in range(4 * q, 4 * q + 4)
                        for n in range(EMB // 512)]
                while fi < len(fill):
                    fill[fi]()
                    fi += 1
                engs = [nc.scalar, nc.sync, nc.gpsimd]
                for j, p in enumerate(pending):
                    p(eng=engs[j % 3])

            for b in range(B):
                nc.vector.memset(QP[b][D:2 * D, 0:SEQ], 0.0)
                nc.vector.memset(QP[b][0:D, SEQ:2 * SEQ], 0.0)
            for u in phase_a_units(0, [0]):
                u()
            nc.sync.dma_start(wout_sb[:], wout.ap())
            phase_bc(0, phase_a_units(1, range(4)),
                     pre={4 * s: phase_a_units(0, [s]) for s in (1, 2, 3)})
            phase_bc(1, [])

    nc.compile()
    return nc


_NC = None


def _get_nc():
    global _NC
    if _NC is None:
        _NC = _build()
    return _NC


def kernel(x, W_qkv, b_qkv, W_out, b_out):
    x = np.asarray(x, dtype=np.float32)
    W_qkv = np.asarray(W_qkv, dtype=np.float32)
    b_qkv = np.asarray(b_qkv, dtype=np.float32)
    W_out = np.asarray(W_out, dtype=np.float32)
    b_out = np.asarray(b_out, dtype=np.float32)

    nc = _get_nc()

    xT = np.ascontiguousarray(
        x.reshape(TSEQ, EMB).T.astype(NPBF16)).reshape(CC, 128, TSEQ)
    Wr = W_qkv.reshape(EMB, 3, HEADS, D)
    br = b_qkv.reshape(3, HEADS, D)
    ones = np.ones((1, D), dtype=np.float32)
    onescol = np.ones((128, 1), dtype=np.float32)

    in_maps = []
    for c in range(NCORES):
        h0, h1 = HPC * c, HPC * (c + 1)
        in_maps.append({
            "xT": xT,
            "wqkv": np.ascontiguousarray(
                np.stack([Wr[:, i, h0:h1].reshape(CC, 128, LD)
                          for i in range(3)], axis=1)
                .transpose(2, 0, 1, 3).reshape(128, CC * 3 * LD)
            ).astype(NPBF16),
            "bqkv": np.ascontiguousarray(
                np.stack([br[i, h0:h1].reshape(LD) for i in range(3)],
                         axis=1)),
            "wout": _round_fp32r(W_out[LD * c:LD * (c + 1)]),
            "ones": ones,
            "onescol": onescol,
        })

    res = bass_utils.run_bass_kernel_spmd(
        nc, in_maps, core_ids=list(range(NCORES)), trace=False)

    acc = np.zeros((TSEQ // 128, 128, EMB), dtype=np.float64)
    for c in range(NCORES):
        acc += res.results[c]["y"]
    out = (acc.reshape(TSEQ, EMB) + b_out).astype(np.float32)
    return out.reshape(B, SEQ, EMB)


# revision 28
# speedup vs baseline: 1.0166x; 1.0166x over previous
"""Multi-head attention (B=2, N=2048, C=1024, H=16) on 8 trn2 NeuronCores.

Tensor-parallel over heads: core c computes heads {2c, 2c+1} for both batch
elements and emits a partial output y_c = attn_out_c @ W_out[local rows];
the host sums the 8 partials and adds b_out.

Per-core pipeline (single TileContext, ~3k instructions, fully unrolled):
  - x^T is loaded once into SBUF (bf16, host pre-transposed/pre-tiled so
    every DMA is a contiguous [128, 512] block; ~600ns trigger cost per DMA
    makes few-and-large transfers matter).
  - QKV^T projection with stacked per-head weights ([128, 128] stationary =
    both heads), biases folded into the PSUM->SBUF eviction.
  - S^T = K @ Q^T with both local heads in full-K=128 matmuls via a
    zero-padded Q^T layout (QP block h holds Q^T_h in its own 64 rows,
    zeros in the other 64: K=64 matmuls measured 424ns vs 222ns for K=128,
    so padding the contraction is a 2x win).
  - P^T = exp(S^T / 32) on ScalarE straight from PSUM ([128, 1024] ops;
    no max-subtraction needed: |scores|/32 <~ 1.5).
  - PV via ones-augmented V (65th stationary column accumulates the softmax
    denominators for free). V is transposed on the PE (128x128 tiles).
  - Normalization: evict PV PSUM, ones-broadcast matmul of the sums row,
    reciprocal_approx_fast (18-bit), DVE multiply -> out^T in fp32r.
  - Output projection in fp32r (fp32 rounded to 11 mantissa bits -- full PE
    rate at free-dim >= 256) with direct DMA of y tiles.

Emission order IS program order for Tile: batch-1 QKV projection units and
deferred projection stores are injected inside batch-0's attention loop
(after their producers) so the static per-engine schedule keeps the PE fed
during exp-gated stretches. Never emit a consumer before its producer:
reads of not-yet-written SBUF regions silently bind to stale contents.

Measured: ~245us on-device (max over 8 cores), PE-bound (~200us PE busy);
absmax error ~4e-3 of output scale vs the fp32 reference (bf16 operand
rounding; projection and normalization run at fp32r precision).
"""
import sys

sys.path.insert(0, "/opt/trn_rl_repo")

import ml_dtypes
import numpy as np

import concourse.bacc as bacc
import concourse.mybir as mybir
import concourse.tile as tile
from concourse import bass_utils
from concourse.masks import make_identity

F32 = mybir.dt.float32
F32R = mybir.dt.float32r
BF16 = mybir.dt.bfloat16
NPBF16 = ml_dtypes.bfloat16

EMB = 1024
HEADS = 16
B = 2
SEQ = 2048
D = 64
NCORES = 8
HPC = HEADS // NCORES          # heads per core = 2
LD = HPC * D                   # local head dim = 128
TSEQ = B * SEQ                 # 4096
CC = EMB // 128                # contraction chunks = 8
SCALE = float(EMB) ** -0.5     # 1/32

QCH = 512                      # q chunk (free dim of S^T matmuls)
NQ = SEQ // QCH                # 4 q-chunks per batch
NK = SEQ // 128                # 16 k-chunks per batch


def _round_fp32r(x: np.ndarray) -> np.ndarray:
    bits = np.ascontiguousarray(x, dtype=np.float32).view(np.uint32)
    out = ((bits.astype(np.uint64) + 0x800) & 0xFFFFF000).astype(np.uint32)
    return out.view(np.float32)


def _build():
    nc = bacc.Bacc("TRN2", target_bir_lowering=False, debug=False,
                   num_devices=NCORES)

    xT = nc.dram_tensor("xT", [CC, 128, TSEQ], BF16, kind="ExternalInput")
    wqkv = nc.dram_tensor("wqkv", [128, CC * 3 * LD], BF16,
                          kind="ExternalInput")
    bqkv = nc.dram_tensor("bqkv", [LD, 3], F32, kind="ExternalInput")
    wout = nc.dram_tensor("wout", [LD, EMB], F32R, kind="ExternalInput")
    ones = nc.dram_tensor("ones", [1, D], F32R, kind="ExternalInput")
    onescol = nc.dram_tensor("onescol", [128, 1], F32R, kind="ExternalInput")
    y = nc.dram_tensor("y", [TSEQ // 128, 128, EMB], F32,
                       kind="ExternalOutput")

    xT_c = xT.ap()
    wqkv_c = wqkv.ap()

    with tile.TileContext(nc) as tc:
        with (
            tc.tile_pool(name="persist", bufs=1) as persist,
            tc.tile_pool(name="xt", bufs=2) as xtp,
            tc.tile_pool(name="vt", bufs=2) as vtp,
            tc.tile_pool(name="psb", bufs=6) as psb,
            tc.tile_pool(name="norm", bufs=3) as normp,
            tc.tile_pool(name="yout", bufs=10) as youtp,
            tc.tile_pool(name="ps_st", bufs=2, space="PSUM") as ps_st,
            tc.tile_pool(name="ps_pv", bufs=1, space="PSUM") as ps_pv,
            tc.tile_pool(name="ps_misc", bufs=2, space="PSUM") as ps_misc,
        ):
            # ---- constants / weights ----
            ident = persist.tile([128, 128], BF16, tag="ident")
            make_identity(nc, ident[:])
            ones_sb = persist.tile([1, D], F32R, tag="ones")
            nc.sync.dma_start(ones_sb[:], ones.ap())
            onescol_sb = persist.tile([128, 1], F32R, tag="onescol")
            nc.sync.dma_start(onescol_sb[:], onescol.ap())

            xfull = {}

            def load_x_window(w):
                for kc in range(CC):
                    t = persist.tile([128, 512], BF16, tag=f"xf{kc}_{w}",
                                     name=f"xf{kc}_{w}")
                    eng = nc.gpsimd if kc % 2 else nc.sync
                    eng.dma_start(t[:], xT_c[kc, :, w * 512:(w + 1) * 512])
                    xfull[kc, w] = t

            load_x_window(0)
            bqkv_sb = persist.tile([LD, 3], F32, tag="bqkv")
            nc.sync.dma_start(bqkv_sb[:], bqkv.ap())
            bias_sb = {nm: bqkv_sb[:, i:i + 1]
                       for i, nm in enumerate(("q", "k", "v"))}
            wall = persist.tile([128, CC * 3 * LD], BF16, tag="wall")
            wchunk = 3 * LD
            for kc in range(CC):
                eng = nc.gpsimd if kc % 2 else nc.sync
                eng.dma_start(wall[:, kc * wchunk:(kc + 1) * wchunk],
                              wqkv_c[:, kc * wchunk:(kc + 1) * wchunk])
            w_sb = {}
            for kc in range(CC):
                for i, nm in enumerate(("q", "k", "v")):
                    w_sb[nm, kc] = wall[:, (kc * 3 + i) * LD:
                                        (kc * 3 + i + 1) * LD]
            for w in range(1, TSEQ // 512):
                load_x_window(w)
            wout_sb = persist.tile([LD, EMB], F32R, tag="wout")

            # persistent activations (per batch)
            # QP: zero-padded Q^T. Block h occupies cols [h*SEQ, (h+1)*SEQ):
            # rows [h*64,(h+1)*64) hold Q^T_h, the other 64 rows are zero.
            QP = [persist.tile([128, HPC * SEQ], BF16, tag=f"QP{b}",
                               name=f"QP{b}") for b in range(B)]
            KT = [persist.tile([LD, SEQ], BF16, tag=f"KT{b}", name=f"KT{b}")
                  for b in range(B)]
            outT = [persist.tile([LD, SEQ], F32R, tag=f"outT{b}",
                                 name=f"outT{b}") for b in range(B)]
            vaug = {}  # (b, kc) -> [128, 2*(D+1)] tile
            for b in range(B):
                for kc in range(NK):
                    vaug[b, kc] = persist.tile([128, 2 * (D + 1)], BF16,
                                               tag=f"vaug{b}_{kc}",
                                               name=f"vaug{b}_{kc}")

            def phase_a_units(b, scs):
                """QKV^T projection + V transpose for batch b, as a list of
                per-tensor emit closures (~2us of PE work each)."""
                units = []
                for sc in scs:
                    s0 = sc * 512            # batch-local seq offset
                    g0 = b * SEQ + s0        # global column in xT

                    def unit(nm, b=b, s0=s0, g0=g0, sc=sc):
                        ps = ps_misc.tile([128, 512], F32, tag="misc")
                        for kc in range(CC):
                            nc.tensor.matmul(
                                ps[:], w_sb[nm, kc],
                                xfull[kc, g0 // 512][:],
                                start=(kc == 0), stop=(kc == CC - 1))
                        if nm == "q":
                            for h in range(HPC):
                                nc.vector.tensor_scalar_add(
                                    QP[b][h * D:(h + 1) * D,
                                          h * SEQ + s0:h * SEQ + s0 + 512],
                                    ps[h * D:(h + 1) * D, :],
                                    bias_sb["q"][h * D:(h + 1) * D, :])
                        elif nm == "k":
                            nc.vector.tensor_scalar_add(
                                KT[b][:, s0:s0 + 512], ps[:], bias_sb["k"])
                        else:
                            vt = vtp.tile([128, 512], BF16, tag="vt")
                            nc.vector.tensor_scalar_add(vt[:], ps[:],
                                                        bias_sb["v"])
                            for j in range(4):
                                va = vaug[b, sc * 4 + j]
                                pst = ps_misc.tile([128, 128], BF16,
                                                   tag="misc")
                                nc.tensor.transpose(
                                    pst[:], vt[:, j * 128:(j + 1) * 128],
                                    ident[:])
                                nc.vector.tensor_copy(va[:, 0:D],
                                                      pst[:, 0:D])
                                nc.vector.tensor_copy(va[:, D + 1:2 * D + 1],
                                                      pst[:, D:2 * D])
                                nc.vector.tensor_copy(va[:, D:D + 1],
                                                      onescol_sb[:])
                                nc.vector.tensor_copy(
                                    va[:, 2 * D + 1:2 * D + 2], onescol_sb[:])

                    for nm in ("q", "k", "v"):
                        units.append(lambda nm=nm, u=unit: u(nm))
                return units

            def phase_bc(b, fill_units, pre=None):
                """Attention for batch b; fill_units and the previous
                q-chunk's projection are injected inside the kc loop so the
                static per-engine order keeps both PE and ACT fed. `pre`
                maps kc -> producer units that must be emitted before that
                kc group of q-chunk 0 (used to overlap the tail of the
                QKV projection with the start of attention)."""
                fill = list(fill_units)
                fi = 0
                pending = []
                pre = pre or {}

                def proj_unit(b, sc, n, eng=None):
                    rt = b * (SEQ // 128) + sc
                    ps = ps_misc.tile([128, 512], F32, tag="misc")
                    nc.tensor.matmul(
                        ps[:], outT[b][:, sc * 128:(sc + 1) * 128],
                        wout_sb[:, n * 512:(n + 1) * 512],
                        start=True, stop=True)
                    yt = youtp.tile([128, 512], F32, tag="yt")
                    nc.vector.tensor_copy(yt[:], ps[:])
                    if eng is None:
                        eng = nc.gpsimd if (sc + n) % 2 else nc.sync
                    eng.dma_start(
                        y.ap()[rt, :, n * 512:(n + 1) * 512], yt[:])

                for q in range(NQ):
                    q0 = q * QCH
                    pvs = [ps_pv.tile([D + 1, QCH], F32, tag=f"pv{h}",
                                      name=f"pv{h}") for h in range(HPC)]
                    for kc in range(NK):
                        if q == 0:
                            for u in pre.get(kc, ()):
                                u()
                        st = ps_st.tile([128, 2 * QCH], F32, tag="st")
                        k0 = kc * 128
                        for h in range(HPC):
                            nc.tensor.matmul(
                                st[:, h * QCH:(h + 1) * QCH],
                                KT[b][:, k0:k0 + 128],
                                QP[b][:, h * SEQ + q0:h * SEQ + q0 + QCH],
                                start=True, stop=True)
                        pt = psb.tile([128, 2 * QCH], BF16, tag="pt")
                        nc.scalar.activation(pt[:], st[:],
                                             mybir.ActivationFunctionType.Exp,
                                             scale=SCALE)
                        for h in range(HPC):
                            nc.tensor.matmul(
                                pvs[h][:],
                                vaug[b, kc][:, h * (D + 1):(h + 1) * (D + 1)],
                                pt[:, h * QCH:(h + 1) * QCH],
                                start=(kc == 0), stop=(kc == NK - 1))
                        if kc % 2 == 1 and pending:
                            pending.pop(0)()
                        if (q > 0 and kc in (2, 5, 8, 11, 14)
                                and fi < len(fill)):
                            fill[fi]()
                            fi += 1
                    # normalize: out^T[d, q] / colsum -> outT (fp32r).
                    for h in range(HPC):
                        pe = normp.tile([D, QCH], F32R, tag="pe")
                        nc.vector.tensor_copy(pe[:], pvs[h][0:D, :])
                        ss = normp.tile([1, QCH], F32R, tag="ss")
                        nc.vector.tensor_copy(ss[:], pvs[h][D:D + 1, :])
                        bc = ps_misc.tile([D, QCH], F32, tag="misc")
                        nc.tensor.matmul(bc[:], ones_sb[:], ss[:],
                                         start=True, stop=True)
                        rc = normp.tile([D, QCH], F32, tag="rc")
                        nc.vector.reciprocal_approx_fast(rc[:], bc[:])
                        nc.vector.tensor_mul(
                            outT[b][h * D:(h + 1) * D, q0:q0 + QCH],
                            pe[:], rc[:])
                    pending += [
                        (lambda b=b, sc=sc, n=n, eng=None:
                         proj_unit(b, sc, n, eng))
                        for sc in range(4 * q, 4 * q + 4)
                        for n in range(EMB // 512)]
                while fi < len(fill):
                    fill[fi]()
                    fi += 1
                engs = [nc.scalar, nc.sync, nc.gpsimd]
                for j, p in enumerate(pending):
                    p(eng=engs[j % 3])

            for b in range(B):
                nc.vector.memset(QP[b][D:2 * D, 0:SEQ], 0.0)
                nc.vector.memset(QP[b][0:D, SEQ:2 * SEQ], 0.0)
            for u in phase_a_units(0, [0]):
                u()
            nc.sync.dma_start(wout_sb[:], wout.ap())
            phase_bc(0, phase_a_units(1, range(4)),
                     pre={4 * s: phase_a_units(0, [s]) for s in (1, 2, 3)})
            phase_bc(1, [])

    nc.compile()
    return nc


_NC = None


def _get_nc():
    global _NC
    if _NC is None:
        _NC = _build()
    return _NC


def kernel(x, W_qkv, b_qkv, W_out, b_out):
    x = np.asarray(x, dtype=np.float32)
    W_qkv = np.asarray(W_qkv, dtype=np.float32)
    b_qkv = np.asarray(b_qkv, dtype=np.float32)
    W_out = np.asarray(W_out, dtype=np.float32)
    b_out = np.asarray(b_out, dtype=np.float32)

    nc = _get_nc()

    xT = np.ascontiguousarray(
        x.reshape(TSEQ, EMB).T.astype(NPBF16)).reshape(CC, 128, TSEQ)
    Wr = W_qkv.reshape(EMB, 3, HEADS, D)
    br = b_qkv.reshape(3, HEADS, D)
    ones = np.ones((1, D), dtype=np.float32)
    onescol = np.ones((128, 1), dtype=np.float32)

    in_maps = []
    for c in range(NCORES):
        h0, h1 = HPC * c, HPC * (c + 1)
        in_maps.append({
            "xT": xT,
            "wqkv": np.ascontiguousarray(
                np.stack([Wr[:, i, h0:h1].reshape(CC, 128, LD)
                          for i in range(3)], axis=1)
                .transpose(2, 0, 1, 3).reshape(128, CC * 3 * LD)
            ).astype(NPBF16),
            "bqkv": np.ascontiguousarray(
                np.stack([br[i, h0:h1].reshape(LD) for i in range(3)],
                         axis=1)),
            "wout": _round_fp32r(W_out[LD * c:LD * (c + 1)]),
            "ones": ones,
            "onescol": onescol,
        })

    res = bass_utils.run_bass_kernel_spmd(
        nc, in_maps, core_ids=list(range(NCORES)), trace=False)

    acc = np.zeros((TSEQ // 128, 128, EMB), dtype=np.float64)
    for c in range(NCORES):
        acc += res.results[c]["y"]
    out = (acc.reshape(TSEQ, EMB) + b_out).astype(np.float32)
    return out.reshape(B, SEQ, EMB)
